# revision 3
# baseline (speedup 1.0000x reference)
"""Trainium2 Bass kernel: batched dense attention (softmax(Q S^T / sqrt(H)) S).

Full problem: query [4, 4096, 1024], source [4, 4096, 1024] (source doubles
as values), output [4, 4096, 1024], all float32.

Sharding: pure data parallel over 8 NeuronCores — core c handles batch
c//2, query rows (c%2)*2048 ... +2048 with the full source for that batch
replicated to the core host-side.  No collectives are needed.

The host pre-casts Q and S to bf16 (the on-chip compute dtype), halving
staging DMA and removing all on-chip casts.

Per-core kernel (flash-attention style, bf16 matmuls, f32 PSUM).  All
transposes run on the DMA xbar engines (not the PE), so the PE does pure
matmul work:
  - S^T staged by 8 DRAM->SBUF xbar DMA-transposes (one per 512-s chunk);
    S natural layout by 32 wide plain DMAs; Q^T tiles by per-qt xbar
    DMA-transposes.
  - per 128-row query tile:
      P1: logits chunks [128q, 512s] = Q^T.T @ S^T  (contract over H)
      exp on ACT with scale=1/32 into per-chunk w tiles, accumulating row
      sums (accum_out)
      W^T via SBUF->SBUF xbar DMA-transpose per 512-chunk
      P2: O' [128q, 512h] += W^T.T @ S  (contract over S), h-halved so a
      single PSUM bank per pass; normalize by the reciprocal row sum
"""

import math

import numpy as np

B, LQ, LS, H = 4, 4096, 4096, 1024
N_CORES = 8
Q_SPLIT = 2  # query-length split within each batch entry
LQ_SH = LQ // Q_SPLIT  # 2048 query rows per core

P = 128  # partitions
SC = 512  # logits chunk width (s columns per P1 matmul / exp call)
OC = 512  # output chunk width (h columns per P2 matmul)
N_WARM = 20  # dummy matmuls to promote the PE HAM clock gate at t=0
QT_PREFETCH = 4  # Q^T tiles staged ahead of the mainline loop


def _build(lq_sh, ls, h):
    """Build + compile the per-core Bass graph for shard shapes."""
    import concourse.bacc as bacc
    import concourse.mybir as mybir
    import concourse.tile as tile

    f32 = mybir.dt.float32
    bf16 = mybir.dt.bfloat16

    n_qt = lq_sh // P  # query tiles
    n_st = ls // P  # source tiles
    n_hc = h // P  # h chunks (contraction tiles for P1)
    sc = min(SC, ls)
    n_sc = ls // sc  # logits chunks
    st_per_sc = sc // P
    oc = min(OC, h)
    n_oc = h // oc  # output chunks
    scale = 1.0 / math.sqrt(h)

    nc = bacc.Bacc(
        "TRN2",
        target_bir_lowering=False,
        debug=False,
        num_devices=N_CORES,
    )
    q_h = nc.dram_tensor("query_input", [lq_sh, h], bf16, kind="ExternalInput")
    s_h = nc.dram_tensor("source_input", [ls, h], bf16, kind="ExternalInput")
    o_h = nc.dram_tensor("out", [lq_sh, h], bf16, kind="ExternalOutput")
    q_ap, s_ap, o_ap = q_h.ap(), s_h.ap(), o_h.ap()

    with tile.TileContext(nc) as tc:
        from contextlib import ExitStack

        with ExitStack() as ctx:
            # PE clock warmup: a dense burst of junk matmuls at t=0 fills the
            # HAM activity window so the 2.4 GHz clock engages before real
            # matmul work arrives (and mainline density then keeps it warm).
            warm_pool = ctx.enter_context(tc.tile_pool(name="warm", bufs=1))
            warm_w = warm_pool.tile([P, P], bf16)
            warm_x = warm_pool.tile([P, sc], bf16)
            nc.vector.memset(warm_w[:], 0.0)
            nc.vector.memset(warm_x[:], 0.0)
            psum_lg = ctx.enter_context(
                tc.tile_pool(name="psum_lg", bufs=4, space="PSUM")
            )
            wp = psum_lg.tile([P, sc], f32, tag="lg", name="warmpsum")
            for _ in range(N_WARM):
                nc.tensor.matmul(wp[:], warm_w[:], warm_x[:], start=True, stop=True)

            persist = ctx.enter_context(tc.tile_pool(name="persist", bufs=1))
            # S^T, split into n_sc chunks.  Chunk sci holds h-chunk hc at
            # cols [hc*sc, +sc): [128h_part, sc s-columns], so P1's moving
            # operand is a contiguous [128, sc] slice.  Filled by one xbar
            # DMA-transpose per chunk: out[p, hc, j] = S[sci*sc + j, hc*P + p].
            s_T = [
                persist.tile([P, n_hc * sc], bf16, tag=f"sT{i}", name=f"sT{i}")
                for i in range(n_sc)
            ]
            # S in natural [s_part, h] layout: tile st at cols [st*h, +h)
            s_nat = persist.tile([P, n_st * h], bf16)

            # S^T first (gates P1 of qt0) on the ACT HWDGE ring.
            for sci in range(n_sc):
                nc.scalar.dma_start(
                    s_T[sci].rearrange("p (hc j) -> p hc j", j=sc),
                    s_ap[sci * sc : (sci + 1) * sc, :],
                    transpose=True,
                )

            qT_pool = ctx.enter_context(tc.tile_pool(name="qT", bufs=QT_PREFETCH + 1))
            w_pool = ctx.enter_context(tc.tile_pool(name="w", bufs=2))
            wT_pool = ctx.enter_context(tc.tile_pool(name="wT", bufs=2))
            r_pool = ctx.enter_context(tc.tile_pool(name="racc", bufs=4))
            psum_o = ctx.enter_context(
                tc.tile_pool(name="psum_o", bufs=4, space="PSUM")
            )
            osb_pool = ctx.enter_context(tc.tile_pool(name="osb", bufs=2))

            # Q^T tiles via xbar DMA-transpose on the SP HWDGE ring:
            # qT[p, hc, j] = Q[qt*P + j, hc*P + p].
            qTs = {}

            def issue_qT(qt):
                t = qT_pool.tile([P, h], bf16, tag="qT")
                # all xbar transposes ride the same (ACT) HWDGE ring: the
                # xbar destination base is engine register state, so
                # transposes on different rings can corrupt each other.
                nc.scalar.dma_start(
                    t.rearrange("p (hc j) -> p hc j", j=P),
                    q_ap[qt * P : (qt + 1) * P, :],
                    transpose=True,
                )
                qTs[qt] = t

            for qt in range(min(QT_PREFETCH, n_qt)):
                issue_qT(qt)

            # s_nat staging (needed from P2 of qt0 onward; SP ring, after
            # the early Q^T tiles).
            for st in range(n_st):
                nc.sync.dma_start(
                    s_nat[:, st * h : (st + 1) * h], s_ap[st * P : (st + 1) * P, :]
                )

            for qt in range(n_qt):
                if qt + QT_PREFETCH < n_qt:
                    issue_qT(qt + QT_PREFETCH)
                qT = qTs.pop(qt)

                # per-chunk exp(logits) tiles and their xbar transposes:
                # wTc[sci][p, b, j] = wc[sci][j, b*P + p]
                wcs = [
                    w_pool.tile([P, sc], bf16, tag=f"wc{i}", name=f"wc{i}")
                    for i in range(n_sc)
                ]
                wTc = [
                    wT_pool.tile([P, sc], bf16, tag=f"wTc{i}", name=f"wTc{i}")
                    for i in range(n_sc)
                ]
                racc = r_pool.tile([P, n_sc], f32, tag="racc")
                for sci in range(n_sc):
                    lg = psum_lg.tile([P, sc], f32, tag="lg")
                    for hc in range(n_hc):
                        nc.tensor.matmul(
                            lg[:],
                            qT[:, hc * P : (hc + 1) * P],
                            s_T[sci][:, hc * sc : (hc + 1) * sc],
                            start=(hc == 0),
                            stop=(hc == n_hc - 1),
                        )
                    nc.scalar.activation(
                        wcs[sci][:],
                        lg[:],
                        mybir.ActivationFunctionType.Exp,
                        scale=scale,
                        accum_out=racc[:, sci : sci + 1],
                    )
                    nc.scalar.dma_start(
                        wTc[sci].rearrange("p (b j) -> p b j", j=P),
                        wcs[sci][:],
                        transpose=True,
                    )

                rsum = r_pool.tile([P, 1], f32, tag="rsum")
                nc.vector.reduce_sum(rsum[:], racc[:], axis=mybir.AxisListType.X)
                rinv = r_pool.tile([P, 1], f32, tag="rinv")
                nc.vector.reciprocal(rinv[:], rsum[:])
                ob = osb_pool.tile([P, h], bf16, tag="ob")
                # h-halved P2: one PSUM bank per output chunk, so two query
                # tiles can have P2 accumulations in flight concurrently.
                for oci in range(n_oc):
                    op = psum_o.tile([P, oc], f32, tag="opsum", name="opsum")
                    for st in range(n_st):
                        sci, b = divmod(st, st_per_sc)
                        nc.tensor.matmul(
                            op[:],
                            wTc[sci][:, b * P : (b + 1) * P],
                            s_nat[:, st * h + oci * oc : st * h + (oci + 1) * oc],
                            start=(st == 0),
                            stop=(st == n_st - 1),
                        )
                    nc.vector.tensor_scalar_mul(
                        ob[:, oci * oc : (oci + 1) * oc], op[:], rinv[:]
                    )
                nc.sync.dma_start(o_ap[qt * P : (qt + 1) * P, :], ob[:])

    nc.compile()
    return nc


_cached_nc = None


def _get_nc():
    global _cached_nc
    if _cached_nc is None:
        _cached_nc = _build(LQ_SH, LS, H)
    return _cached_nc


def _in_maps(query_input, source_input):
    import ml_dtypes

    bf16 = ml_dtypes.bfloat16
    q = np.asarray(query_input, dtype=np.float32).astype(bf16)
    s = np.asarray(source_input, dtype=np.float32).astype(bf16)
    assert q.shape == (B, LQ, H) and s.shape == (B, LS, H)
    in_maps = []
    for c in range(N_CORES):
        b, qh = divmod(c, Q_SPLIT)
        in_maps.append(
            {
                "query_input": np.ascontiguousarray(
                    q[b, qh * LQ_SH : (qh + 1) * LQ_SH, :]
                ),
                "source_input": np.ascontiguousarray(s[b]),
            }
        )
    return in_maps


def _gather(results):
    out = np.empty((B, LQ, H), dtype=np.float32)
    for c in range(N_CORES):
        b, qh = divmod(c, Q_SPLIT)
        out[b, qh * LQ_SH : (qh + 1) * LQ_SH, :] = results[c]["out"]
    return out


def kernel(query_input, source_input):
    from concourse.bass_utils import run_bass_kernel_spmd

    res = run_bass_kernel_spmd(
        _get_nc(),
        _in_maps(query_input, source_input),
        core_ids=list(range(N_CORES)),
    )
    return _gather(res.results)


# revision 16
# speedup vs baseline: 1.0614x; 1.0614x over previous
"""Trainium2 Bass kernel: batched dense attention (softmax(Q S^T / sqrt(H)) S).

Full problem: query [4, 4096, 1024], source [4, 4096, 1024] (source doubles
as values), output [4, 4096, 1024], all float32.

Sharding: pure data parallel over 8 NeuronCores — core c handles batch
c//2, query rows (c%2)*2048 ... +2048 with the full source for that batch
replicated to the core host-side.  No collectives are needed.

The host pre-casts Q and S to bf16 (the on-chip compute dtype), halving
staging DMA and removing all on-chip casts.

Per-core kernel (transposed-P1 formulation, bf16 matmuls, f32 PSUM).  The
PE does pure matmul work — no PE transposes at all:
  - S^T staged by 8 DRAM->SBUF xbar DMA-transposes; Q^T by one xbar
    DMA-transpose per 512-query block.  All xbar transposes ride the same
    (ACT) HWDGE ring — the xbar destination base is engine register state,
    so transposes on different rings corrupt each other.
  - S natural layout staged with a leading all-ones column per source
    tile ([128, 1+h] each), so P2 computes softmax row sums for free.
  - per 512-query block:
      P1T: logitsT tiles [128s, 512q] = S^T_tile.T @ Q^T  (contract over H)
      exp on ACT with scale=1/32 -> W^T tiles [128s, 512q] in SBUF, which
      are directly P2's stationary operand (this is why P1 is transposed:
      no W transpose is ever needed, and no softmax-sum accumulation).
      P2 per 128-query tile: O'[q, {R, h}] += W^T.T @ [1|S] over 3 PSUM
      chunks (342+342+341 cols); chunk 0 col 0 is the row sum R.
      Normalize by 1/R on DVE while writing bf16 output.
"""

import math

import numpy as np

B, LQ, LS, H = 4, 4096, 4096, 1024
N_CORES = 8
Q_SPLIT = 2  # query-length split within each batch entry
LQ_SH = LQ // Q_SPLIT  # 2048 query rows per core

P = 128  # partitions
QB = 512  # query-block width (P1T moving columns)
SC = 512  # S^T staging chunk width (s columns per xbar transpose)
N_WARM = 30  # dummy matmuls to promote the PE HAM clock gate at t=0
# P2 output chunking: 1 (ones) + 1024 (h) columns in 3 PSUM chunks
P2_CHUNKS = ((0, 342), (342, 342), (684, 341))


def _build(lq_sh, ls, h):
    """Build + compile the per-core Bass graph for shard shapes."""
    import concourse.bacc as bacc
    import concourse.mybir as mybir
    import concourse.tile as tile

    f32 = mybir.dt.float32
    bf16 = mybir.dt.bfloat16

    n_st = ls // P  # source tiles
    n_hc = h // P  # h chunks (contraction tiles for P1T)
    sc = min(SC, ls)
    n_sc = ls // sc  # S^T staging chunks
    st_per_sc = sc // P
    qb = min(QB, lq_sh)
    n_qb = lq_sh // qb  # query blocks
    qt_per_qb = qb // P
    w = h + 1  # per-source-tile staged width (ones column + h)
    scale = 1.0 / math.sqrt(h)

    nc = bacc.Bacc(
        "TRN2",
        target_bir_lowering=False,
        debug=False,
        num_devices=N_CORES,
    )
    qt_h = nc.dram_tensor("q_t", [h, lq_sh], bf16, kind="ExternalInput")
    s_h = nc.dram_tensor("source_input", [ls, h], bf16, kind="ExternalInput")
    st_h = nc.dram_tensor("s_t", [h, ls], bf16, kind="ExternalInput")
    o_h = nc.dram_tensor("out", [lq_sh, h], bf16, kind="ExternalOutput")
    s_ap, o_ap = s_h.ap(), o_h.ap()
    # [h, n] DRAM views as [p, hc, n]: row hc*P + p
    qt_ap3 = qt_h.ap().rearrange("(hc p) n -> p hc n", p=P)
    st_ap3 = st_h.ap().rearrange("(hc p) n -> p hc n", p=P)

    with tile.TileContext(nc) as tc:
        from contextlib import ExitStack

        with ExitStack() as ctx:
            # PE clock warmup: a dense burst of junk matmuls at t=0 fills the
            # HAM activity window so the 2.4 GHz clock engages before real
            # matmul work arrives (and mainline density then keeps it warm).
            warm_pool = ctx.enter_context(tc.tile_pool(name="warm", bufs=1))
            warm_w = warm_pool.tile([P, P], bf16)
            warm_x = warm_pool.tile([P, qb], bf16)
            nc.vector.memset(warm_w[:], 0.0)
            nc.vector.memset(warm_x[:], 0.0)
            psum_lg = ctx.enter_context(
                tc.tile_pool(name="psum_lg", bufs=4, space="PSUM")
            )
            wp = psum_lg.tile([P, qb], f32, tag="lgT", name="warmpsum")
            for _ in range(N_WARM):
                nc.tensor.matmul(wp[:], warm_w[:], warm_x[:], start=True, stop=True)

            qT_pool = ctx.enter_context(tc.tile_pool(name="qT", bufs=2))
            qTs = {}

            def issue_qT(b):
                # qT[p, hc, j] = Q^T[hc*P + p, b*qb + j], plain DMA
                t = qT_pool.tile([P, n_hc * qb], bf16, tag="qT")
                nc.sync.dma_start(
                    t.rearrange("p (hc j) -> p hc j", j=qb),
                    qt_ap3[:, :, b * qb : (b + 1) * qb],
                )
                qTs[b] = t

            issue_qT(0)

            persist = ctx.enter_context(tc.tile_pool(name="persist", bufs=1))
            # S^T in n_sc chunks; chunk sci holds h-chunk hc at cols
            # [hc*sc, +sc): sT[p, hc*sc + j] = S[sci*sc + j, hc*P + p].
            s_T = [
                persist.tile([P, n_hc * sc], bf16, tag=f"sT{i}", name=f"sT{i}")
                for i in range(n_sc)
            ]
            for sci in range(n_sc):
                nc.sync.dma_start(
                    s_T[sci].rearrange("p (hc j) -> p hc j", j=sc),
                    st_ap3[:, :, sci * sc : (sci + 1) * sc],
                )

            # S natural layout with a leading ones column per source tile:
            # tile st at cols [st*w, +w): col 0 = 1.0, cols 1..h = S[st*P+p, :].
            s_nat = persist.tile([P, n_st * w], bf16)
            nc.vector.memset(
                s_nat.rearrange("p (st c) -> p st c", c=w)[:, :, 0:1], 1.0
            )
            for st in range(n_st):
                nc.sync.dma_start(
                    s_nat[:, st * w + 1 : (st + 1) * w],
                    s_ap[st * P : (st + 1) * P, :],
                )

            # W^T tiles: one [128s, qb] tile per source tile, written by ACT
            # exp directly (the transposed-P1 trick).  Single-buffered: the
            # PE's own P2(k) -> P1T(k+1) ordering provides the reuse window.
            wT_pool = ctx.enter_context(tc.tile_pool(name="wT", bufs=1))
            psum_o = ctx.enter_context(
                tc.tile_pool(name="psum_o", bufs=4, space="PSUM")
            )
            r_pool = ctx.enter_context(tc.tile_pool(name="r", bufs=8))
            osb_pool = ctx.enter_context(tc.tile_pool(name="osb", bufs=3))

            for b in range(n_qb):
                if b + 1 < n_qb:
                    issue_qT(b + 1)
                qT = qTs.pop(b)
                wT = [
                    wT_pool.tile([P, qb], bf16, tag=f"wt{st}", name=f"wt{st}")
                    for st in range(n_st)
                ]
                # P1T: logitsT tiles, one per source tile
                for st in range(n_st):
                    sci, soff = divmod(st, st_per_sc)
                    lgT = psum_lg.tile([P, qb], f32, tag="lgT")
                    for hc in range(n_hc):
                        nc.tensor.matmul(
                            lgT[:],
                            s_T[sci][
                                :, hc * sc + soff * P : hc * sc + (soff + 1) * P
                            ],
                            qT[:, hc * qb : (hc + 1) * qb],
                            start=(hc == 0),
                            stop=(hc == n_hc - 1),
                        )
                    nc.scalar.activation(
                        wT[st][:],
                        lgT[:],
                        mybir.ActivationFunctionType.Exp,
                        scale=scale,
                    )

                # P2 per 128-query tile: 3 chunks over [1|S]; chunk 0 col 0
                # accumulates the softmax row sum.
                for qs in range(qt_per_qb):
                    ob = osb_pool.tile([P, h], bf16, tag="ob")
                    rinv = r_pool.tile([P, 1], f32, tag="rinv")
                    for ci, (coff, cw) in enumerate(P2_CHUNKS):
                        opt = psum_o.tile([P, P2_CHUNKS[0][1]], f32, tag="op", name="op")
                        op = opt[:, :cw]
                        for st in range(n_st):
                            nc.tensor.matmul(
                                op,
                                wT[st][:, qs * P : (qs + 1) * P],
                                s_nat[:, st * w + coff : st * w + coff + cw],
                                start=(st == 0),
                                stop=(st == n_st - 1),
                            )
                        if ci == 0:
                            nc.vector.reciprocal(rinv[:], op[:, 0:1])
                            nc.vector.tensor_scalar_mul(
                                ob[:, 0 : cw - 1], op[:, 1:cw], rinv[:]
                            )
                        else:
                            nc.vector.tensor_scalar_mul(
                                ob[:, coff - 1 : coff - 1 + cw], op, rinv[:]
                            )
                        # store each chunk as soon as it is normalized: the
                        # final tile drains during the last chunks' matmuls
                        lo = 0 if ci == 0 else coff - 1
                        hi = coff - 1 + cw
                        qrow = (b * qt_per_qb + qs) * P
                        nc.sync.dma_start(
                            o_ap[qrow : qrow + P, lo:hi], ob[:, lo:hi]
                        )

    nc.compile()
    return nc


_cached_nc = None


def _get_nc():
    global _cached_nc
    if _cached_nc is None:
        _cached_nc = _build(LQ_SH, LS, H)
    return _cached_nc


def _in_maps(query_input, source_input):
    import ml_dtypes

    bf16 = ml_dtypes.bfloat16
    q = np.asarray(query_input, dtype=np.float32).astype(bf16)
    s = np.asarray(source_input, dtype=np.float32).astype(bf16)
    assert q.shape == (B, LQ, H) and s.shape == (B, LS, H)
    in_maps = []
    for c in range(N_CORES):
        b, qh = divmod(c, Q_SPLIT)
        in_maps.append(
            {
                "q_t": np.ascontiguousarray(
                    q[b, qh * LQ_SH : (qh + 1) * LQ_SH, :].T
                ),
                "source_input": np.ascontiguousarray(s[b]),
                "s_t": np.ascontiguousarray(s[b].T),
            }
        )
    return in_maps


def _gather(results):
    out = np.empty((B, LQ, H), dtype=np.float32)
    for c in range(N_CORES):
        b, qh = divmod(c, Q_SPLIT)
        out[b, qh * LQ_SH : (qh + 1) * LQ_SH, :] = results[c]["out"]
    return out


def kernel(query_input, source_input):
    from concourse.bass_utils import run_bass_kernel_spmd

    res = run_bass_kernel_spmd(
        _get_nc(),
        _in_maps(query_input, source_input),
        core_ids=list(range(N_CORES)),
    )
    return _gather(res.results)


# revision 17
# speedup vs baseline: 1.0702x; 1.0083x over previous
"""Trainium2 Bass kernel: batched dense attention (softmax(Q S^T / sqrt(H)) S).

Full problem: query [4, 4096, 1024], source [4, 4096, 1024] (source doubles
as values), output [4, 4096, 1024], all float32.

Sharding: pure data parallel over 8 NeuronCores — core c handles batch
c//2, query rows (c%2)*2048 ... +2048 with the full source for that batch
replicated to the core host-side.  No collectives are needed.

The host pre-casts Q and S to bf16 (the on-chip compute dtype), halving
staging DMA and removing all on-chip casts.

Per-core kernel (transposed-P1 formulation, bf16 matmuls, f32 PSUM).  The
PE does pure matmul work — no PE transposes at all:
  - S^T staged by 8 DRAM->SBUF xbar DMA-transposes; Q^T by one xbar
    DMA-transpose per 512-query block.  All xbar transposes ride the same
    (ACT) HWDGE ring — the xbar destination base is engine register state,
    so transposes on different rings corrupt each other.
  - S natural layout staged with a leading all-ones column per source
    tile ([128, 1+h] each), so P2 computes softmax row sums for free.
  - per 512-query block:
      P1T: logitsT tiles [128s, 512q] = S^T_tile.T @ Q^T  (contract over H)
      exp on ACT with scale=1/32 -> W^T tiles [128s, 512q] in SBUF, which
      are directly P2's stationary operand (this is why P1 is transposed:
      no W transpose is ever needed, and no softmax-sum accumulation).
      P2 per 128-query tile: O'[q, {R, h}] += W^T.T @ [1|S] over 3 PSUM
      chunks (342+342+341 cols); chunk 0 col 0 is the row sum R.
      Normalize by 1/R on DVE while writing bf16 output.
"""

import math

import numpy as np

B, LQ, LS, H = 4, 4096, 4096, 1024
N_CORES = 8
Q_SPLIT = 2  # query-length split within each batch entry
LQ_SH = LQ // Q_SPLIT  # 2048 query rows per core

P = 128  # partitions
QB = 512  # query-block width (P1T moving columns)
SC = 512  # S^T staging chunk width (s columns per xbar transpose)
N_WARM = 16  # dummy matmuls to promote the PE HAM clock gate at t=0
# P2 output chunking: 1 (ones) + 1024 (h) columns in 3 PSUM chunks
P2_CHUNKS = ((0, 342), (342, 342), (684, 341))


def _build(lq_sh, ls, h):
    """Build + compile the per-core Bass graph for shard shapes."""
    import concourse.bacc as bacc
    import concourse.mybir as mybir
    import concourse.tile as tile

    f32 = mybir.dt.float32
    bf16 = mybir.dt.bfloat16

    n_st = ls // P  # source tiles
    n_hc = h // P  # h chunks (contraction tiles for P1T)
    sc = min(SC, ls)
    n_sc = ls // sc  # S^T staging chunks
    st_per_sc = sc // P
    qb = min(QB, lq_sh)
    n_qb = lq_sh // qb  # query blocks
    qt_per_qb = qb // P
    w = h + 1  # per-source-tile staged width (ones column + h)
    scale = 1.0 / math.sqrt(h)

    nc = bacc.Bacc(
        "TRN2",
        target_bir_lowering=False,
        debug=False,
        num_devices=N_CORES,
    )
    qt_h = nc.dram_tensor("q_t", [h, lq_sh], bf16, kind="ExternalInput")
    s_h = nc.dram_tensor("source_input", [ls, h], bf16, kind="ExternalInput")
    st_h = nc.dram_tensor("s_t", [h, ls], bf16, kind="ExternalInput")
    o_h = nc.dram_tensor("out", [lq_sh, h], bf16, kind="ExternalOutput")
    s_ap, o_ap = s_h.ap(), o_h.ap()
    # [h, n] DRAM views as [p, hc, n]: row hc*P + p
    qt_ap3 = qt_h.ap().rearrange("(hc p) n -> p hc n", p=P)
    st_ap3 = st_h.ap().rearrange("(hc p) n -> p hc n", p=P)

    with tile.TileContext(nc) as tc:
        from contextlib import ExitStack

        with ExitStack() as ctx:
            # PE clock warmup: a dense burst of junk matmuls at t=0 fills the
            # HAM activity window so the 2.4 GHz clock engages before real
            # matmul work arrives (and mainline density then keeps it warm).
            warm_pool = ctx.enter_context(tc.tile_pool(name="warm", bufs=1))
            warm_w = warm_pool.tile([P, P], bf16)
            warm_x = warm_pool.tile([P, qb], bf16)
            nc.vector.memset(warm_w[:], 0.0)
            nc.vector.memset(warm_x[:], 0.0)
            psum_lg = ctx.enter_context(
                tc.tile_pool(name="psum_lg", bufs=4, space="PSUM")
            )
            wp = psum_lg.tile([P, qb], f32, tag="lgT", name="warmpsum")
            for _ in range(N_WARM):
                nc.tensor.matmul(wp[:], warm_w[:], warm_x[:], start=True, stop=True)

            qT_pool = ctx.enter_context(tc.tile_pool(name="qT", bufs=2))
            qTs = {}

            def issue_qT(b):
                # qT[p, hc, j] = Q^T[hc*P + p, b*qb + j], plain DMA
                t = qT_pool.tile([P, n_hc * qb], bf16, tag="qT")
                nc.sync.dma_start(
                    t.rearrange("p (hc j) -> p hc j", j=qb),
                    qt_ap3[:, :, b * qb : (b + 1) * qb],
                )
                qTs[b] = t

            issue_qT(0)

            persist = ctx.enter_context(tc.tile_pool(name="persist", bufs=1))
            # S^T in n_sc chunks; chunk sci holds h-chunk hc at cols
            # [hc*sc, +sc): sT[p, hc*sc + j] = S[sci*sc + j, hc*P + p].
            # Chunk 0 is staged as st_per_sc per-source-tile tiles instead, so
            # the first P1T chain's stationary (256 KB) lands ~4us earlier
            # than a whole 1 MiB chunk would.
            s_T2 = [
                persist.tile([P, n_hc * P], bf16, tag=f"sT2_{i}", name=f"sT2_{i}")
                for i in range(st_per_sc)
            ]
            s_T = {
                i: persist.tile([P, n_hc * sc], bf16, tag=f"sT{i}", name=f"sT{i}")
                for i in range(1, n_sc)
            }
            for st in range(st_per_sc):
                nc.sync.dma_start(
                    s_T2[st].rearrange("p (hc j) -> p hc j", j=P),
                    st_ap3[:, :, st * P : (st + 1) * P],
                )
            for sci in range(1, n_sc):
                nc.sync.dma_start(
                    s_T[sci].rearrange("p (hc j) -> p hc j", j=sc),
                    st_ap3[:, :, sci * sc : (sci + 1) * sc],
                )

            # S natural layout with a leading ones column per source tile:
            # tile st at cols [st*w, +w): col 0 = 1.0, cols 1..h = S[st*P+p, :].
            s_nat = persist.tile([P, n_st * w], bf16)
            nc.vector.memset(
                s_nat.rearrange("p (st c) -> p st c", c=w)[:, :, 0:1], 1.0
            )
            for st in range(n_st):
                nc.sync.dma_start(
                    s_nat[:, st * w + 1 : (st + 1) * w],
                    s_ap[st * P : (st + 1) * P, :],
                )

            # W^T tiles: one [128s, qb] tile per source tile, written by ACT
            # exp directly (the transposed-P1 trick).  Single-buffered: the
            # PE's own P2(k) -> P1T(k+1) ordering provides the reuse window.
            wT_pool = ctx.enter_context(tc.tile_pool(name="wT", bufs=1))
            psum_o = ctx.enter_context(
                tc.tile_pool(name="psum_o", bufs=4, space="PSUM")
            )
            r_pool = ctx.enter_context(tc.tile_pool(name="r", bufs=8))
            osb_pool = ctx.enter_context(tc.tile_pool(name="osb", bufs=3))

            for b in range(n_qb):
                if b + 1 < n_qb:
                    issue_qT(b + 1)
                qT = qTs.pop(b)
                wT = [
                    wT_pool.tile([P, qb], bf16, tag=f"wt{st}", name=f"wt{st}")
                    for st in range(n_st)
                ]
                # P1T: logitsT tiles, one per source tile
                for st in range(n_st):
                    sci, soff = divmod(st, st_per_sc)
                    lgT = psum_lg.tile([P, qb], f32, tag="lgT")
                    for hc in range(n_hc):
                        if sci == 0:
                            stat = s_T2[st][:, hc * P : (hc + 1) * P]
                        else:
                            stat = s_T[sci][
                                :, hc * sc + soff * P : hc * sc + (soff + 1) * P
                            ]
                        nc.tensor.matmul(
                            lgT[:],
                            stat,
                            qT[:, hc * qb : (hc + 1) * qb],
                            start=(hc == 0),
                            stop=(hc == n_hc - 1),
                        )
                    nc.scalar.activation(
                        wT[st][:],
                        lgT[:],
                        mybir.ActivationFunctionType.Exp,
                        scale=scale,
                    )

                # P2 per 128-query tile: 3 chunks over [1|S]; chunk 0 col 0
                # accumulates the softmax row sum.
                for qs in range(qt_per_qb):
                    ob = osb_pool.tile([P, h], bf16, tag="ob")
                    rinv = r_pool.tile([P, 1], f32, tag="rinv")
                    for ci, (coff, cw) in enumerate(P2_CHUNKS):
                        opt = psum_o.tile([P, P2_CHUNKS[0][1]], f32, tag="op", name="op")
                        op = opt[:, :cw]
                        for st in range(n_st):
                            nc.tensor.matmul(
                                op,
                                wT[st][:, qs * P : (qs + 1) * P],
                                s_nat[:, st * w + coff : st * w + coff + cw],
                                start=(st == 0),
                                stop=(st == n_st - 1),
                            )
                        if ci == 0:
                            nc.vector.reciprocal(rinv[:], op[:, 0:1])
                            nc.vector.tensor_scalar_mul(
                                ob[:, 0 : cw - 1], op[:, 1:cw], rinv[:]
                            )
                        else:
                            nc.vector.tensor_scalar_mul(
                                ob[:, coff - 1 : coff - 1 + cw], op, rinv[:]
                            )
                        # store each chunk as soon as it is normalized: the
                        # final tile drains during the last chunks' matmuls
                        lo = 0 if ci == 0 else coff - 1
                        hi = coff - 1 + cw
                        qrow = (b * qt_per_qb + qs) * P
                        nc.sync.dma_start(
                            o_ap[qrow : qrow + P, lo:hi], ob[:, lo:hi]
                        )

    nc.compile()
    return nc


_cached_nc = None


def _get_nc():
    global _cached_nc
    if _cached_nc is None:
        _cached_nc = _build(LQ_SH, LS, H)
    return _cached_nc


def _in_maps(query_input, source_input):
    import ml_dtypes

    bf16 = ml_dtypes.bfloat16
    q = np.asarray(query_input, dtype=np.float32).astype(bf16)
    s = np.asarray(source_input, dtype=np.float32).astype(bf16)
    assert q.shape == (B, LQ, H) and s.shape == (B, LS, H)
    in_maps = []
    for c in range(N_CORES):
        b, qh = divmod(c, Q_SPLIT)
        in_maps.append(
            {
                "q_t": np.ascontiguousarray(
                    q[b, qh * LQ_SH : (qh + 1) * LQ_SH, :].T
                ),
                "source_input": np.ascontiguousarray(s[b]),
                "s_t": np.ascontiguousarray(s[b].T),
            }
        )
    return in_maps


def _gather(results):
    out = np.empty((B, LQ, H), dtype=np.float32)
    for c in range(N_CORES):
        b, qh = divmod(c, Q_SPLIT)
        out[b, qh * LQ_SH : (qh + 1) * LQ_SH, :] = results[c]["out"]
    return out


def kernel(query_input, source_input):
    from concourse.bass_utils import run_bass_kernel_spmd

    res = run_bass_kernel_spmd(
        _get_nc(),
        _in_maps(query_input, source_input),
        core_ids=list(range(N_CORES)),
    )
    return _gather(res.results)


# revision 18
# speedup vs baseline: 1.2715x; 1.1881x over previous
"""Trainium2 Bass kernel: batched dense attention (softmax(Q S^T / sqrt(H)) S).

Full problem: query [4, 4096, 1024], source [4, 4096, 1024] (source doubles
as values), output [4, 4096, 1024], all float32.

Sharding: pure data parallel over 8 NeuronCores — core c handles batch
c//2, query rows (c%2)*2048 ... +2048 with the full source for that batch
replicated to the core host-side.  No collectives are needed.

The host pre-casts Q and S to bf16 (the on-chip compute dtype), halving
staging DMA and removing all on-chip casts.

Per-core kernel (transposed-P1 formulation, bf16 matmuls, f32 PSUM).  The
PE does pure matmul work — no PE transposes at all:
  - S^T staged by 8 DRAM->SBUF xbar DMA-transposes; Q^T by one xbar
    DMA-transpose per 512-query block.  All xbar transposes ride the same
    (ACT) HWDGE ring — the xbar destination base is engine register state,
    so transposes on different rings corrupt each other.
  - S natural layout staged with a leading all-ones column per source
    tile ([128, 1+h] each), so P2 computes softmax row sums for free.
  - per 512-query block:
      P1T: logitsT tiles [128s, 512q] = S^T_tile.T @ Q^T  (contract over H)
      exp on ACT with scale=1/32 -> W^T tiles [128s, 512q] in SBUF, which
      are directly P2's stationary operand (this is why P1 is transposed:
      no W transpose is ever needed, and no softmax-sum accumulation).
      P2 per 128-query tile: O'[q, {R, h}] += W^T.T @ [1|S] over 3 PSUM
      chunks (342+342+341 cols); chunk 0 col 0 is the row sum R.
      Normalize by 1/R on DVE while writing bf16 output.
"""

import math

import numpy as np

B, LQ, LS, H = 4, 4096, 4096, 1024
N_CORES = 8
Q_SPLIT = 2  # query-length split within each batch entry
LQ_SH = LQ // Q_SPLIT  # 2048 query rows per core

P = 128  # partitions
QB = 512  # query-block width (P1T moving columns)
SC = 512  # S^T staging chunk width (s columns per xbar transpose)
N_WARM = 9  # dummy matmuls to promote the PE HAM clock gate at t=0
# P2 output chunking: 1 (ones) + 1024 (h) columns in 3 PSUM chunks
P2_CHUNKS = ((0, 342), (342, 342), (684, 341))


def _build(lq_sh, ls, h):
    """Build + compile the per-core Bass graph for shard shapes."""
    import concourse.bacc as bacc
    import concourse.mybir as mybir
    import concourse.tile as tile

    f32 = mybir.dt.float32
    bf16 = mybir.dt.bfloat16

    n_st = ls // P  # source tiles
    n_hc = h // P  # h chunks (contraction tiles for P1T)
    sc = min(SC, ls)
    n_sc = ls // sc  # S^T staging chunks
    st_per_sc = sc // P
    qb = min(QB, lq_sh)
    n_qb = lq_sh // qb  # query blocks
    qt_per_qb = qb // P
    w = h + 1  # per-source-tile staged width (ones column + h)
    scale = 1.0 / math.sqrt(h)

    nc = bacc.Bacc(
        "TRN2",
        target_bir_lowering=False,
        debug=False,
        num_devices=N_CORES,
    )
    qt_h = nc.dram_tensor("q_t", [h, lq_sh], bf16, kind="ExternalInput")
    s_h = nc.dram_tensor("source_input", [ls, h], bf16, kind="ExternalInput")
    st_h = nc.dram_tensor("s_t", [h, ls], bf16, kind="ExternalInput")
    o_h = nc.dram_tensor("out", [lq_sh, h], bf16, kind="ExternalOutput")
    s_ap, o_ap = s_h.ap(), o_h.ap()
    # [h, n] DRAM views as [p, hc, n]: row hc*P + p
    qt_ap3 = qt_h.ap().rearrange("(hc p) n -> p hc n", p=P)
    st_ap3 = st_h.ap().rearrange("(hc p) n -> p hc n", p=P)

    with tile.TileContext(nc) as tc:
        from contextlib import ExitStack

        with ExitStack() as ctx:
            # PE clock warmup: a dense burst of junk matmuls at t=0 fills the
            # HAM activity window so the 2.4 GHz clock engages before real
            # matmul work arrives (and mainline density then keeps it warm).
            warm_pool = ctx.enter_context(tc.tile_pool(name="warm", bufs=1))
            warm_w = warm_pool.tile([P, P], bf16)
            warm_x = warm_pool.tile([P, qb], bf16)
            nc.vector.memset(warm_w[:], 0.0)
            nc.vector.memset(warm_x[:], 0.0)
            psum_lg = ctx.enter_context(
                tc.tile_pool(name="psum_lg", bufs=4, space="PSUM")
            )
            wp = psum_lg.tile([P, qb], f32, tag="lgT", name="warmpsum")
            for _ in range(N_WARM):
                nc.tensor.matmul(wp[:], warm_w[:], warm_x[:], start=True, stop=True)

            qT_pool = ctx.enter_context(tc.tile_pool(name="qT", bufs=2))
            qTs = {}

            def issue_qT(b):
                # qT[p, hc, j] = Q^T[hc*P + p, b*qb + j], plain DMA.  Block 0
                # is split in two hc-halves so the first P1T chains can start
                # on hc 0-3 while hc 4-7 is still in flight.
                t = qT_pool.tile([P, n_hc * qb], bf16, tag="qT")
                t3 = t.rearrange("p (hc j) -> p hc j", j=qb)
                if b == 0:
                    half = n_hc // 2
                    for g in range(2):
                        nc.sync.dma_start(
                            t3[:, g * half : (g + 1) * half, :],
                            qt_ap3[:, g * half : (g + 1) * half, b * qb : (b + 1) * qb],
                        )
                else:
                    nc.sync.dma_start(t3, qt_ap3[:, :, b * qb : (b + 1) * qb])
                qTs[b] = t

            issue_qT(0)

            persist = ctx.enter_context(tc.tile_pool(name="persist", bufs=1))
            # S^T in n_sc chunks; chunk sci holds h-chunk hc at cols
            # [hc*sc, +sc): sT[p, hc*sc + j] = S[sci*sc + j, hc*P + p].
            # Chunk 0 is staged as st_per_sc per-source-tile tiles instead, so
            # the first P1T chain's stationary (256 KB) lands ~4us earlier
            # than a whole 1 MiB chunk would.
            s_T2 = [
                persist.tile([P, n_hc * P], bf16, tag=f"sT2_{i}", name=f"sT2_{i}")
                for i in range(st_per_sc)
            ]
            s_T = {
                i: persist.tile([P, n_hc * sc], bf16, tag=f"sT{i}", name=f"sT{i}")
                for i in range(1, n_sc)
            }
            for st in range(st_per_sc):
                nc.sync.dma_start(
                    s_T2[st].rearrange("p (hc j) -> p hc j", j=P),
                    st_ap3[:, :, st * P : (st + 1) * P],
                )
            for sci in range(1, n_sc):
                nc.sync.dma_start(
                    s_T[sci].rearrange("p (hc j) -> p hc j", j=sc),
                    st_ap3[:, :, sci * sc : (sci + 1) * sc],
                )

            # S natural layout with a leading ones column per source tile:
            # tile st at cols [st*w, +w): col 0 = 1.0, cols 1..h = S[st*P+p, :].
            s_nat = persist.tile([P, n_st * w], bf16)
            nc.vector.memset(
                s_nat.rearrange("p (st c) -> p st c", c=w)[:, :, 0:1], 1.0
            )
            for st in range(n_st):
                nc.sync.dma_start(
                    s_nat[:, st * w + 1 : (st + 1) * w],
                    s_ap[st * P : (st + 1) * P, :],
                )

            # W^T tiles: one [128s, qb] tile per source tile, written by ACT
            # exp directly (the transposed-P1 trick).  Single-buffered: the
            # PE's own P2(k) -> P1T(k+1) ordering provides the reuse window.
            wT_pool = ctx.enter_context(tc.tile_pool(name="wT", bufs=1))
            psum_o = ctx.enter_context(
                tc.tile_pool(name="psum_o", bufs=4, space="PSUM")
            )
            r_pool = ctx.enter_context(tc.tile_pool(name="r", bufs=8))
            osb_pool = ctx.enter_context(tc.tile_pool(name="osb", bufs=3))

            for b in range(n_qb):
                if b + 1 < n_qb:
                    issue_qT(b + 1)
                qT = qTs.pop(b)
                wT = [
                    wT_pool.tile([P, qb], bf16, tag=f"wt{st}", name=f"wt{st}")
                    for st in range(n_st)
                ]
                # P1T: logitsT tiles, one per source tile
                for st in range(n_st):
                    sci, soff = divmod(st, st_per_sc)
                    lgT = psum_lg.tile([P, qb], f32, tag="lgT")
                    for hc in range(n_hc):
                        if sci == 0:
                            stat = s_T2[st][:, hc * P : (hc + 1) * P]
                        else:
                            stat = s_T[sci][
                                :, hc * sc + soff * P : hc * sc + (soff + 1) * P
                            ]
                        nc.tensor.matmul(
                            lgT[:],
                            stat,
                            qT[:, hc * qb : (hc + 1) * qb],
                            start=(hc == 0),
                            stop=(hc == n_hc - 1),
                        )
                    nc.scalar.activation(
                        wT[st][:],
                        lgT[:],
                        mybir.ActivationFunctionType.Exp,
                        scale=scale,
                    )

                # P2 per 128-query tile: 3 chunks over [1|S]; chunk 0 col 0
                # accumulates the softmax row sum.
                for qs in range(qt_per_qb):
                    ob = osb_pool.tile([P, h], bf16, tag="ob")
                    rinv = r_pool.tile([P, 1], f32, tag="rinv")
                    for ci, (coff, cw) in enumerate(P2_CHUNKS):
                        opt = psum_o.tile([P, P2_CHUNKS[0][1]], f32, tag="op", name="op")
                        op = opt[:, :cw]
                        for st in range(n_st):
                            nc.tensor.matmul(
                                op,
                                wT[st][:, qs * P : (qs + 1) * P],
                                s_nat[:, st * w + coff : st * w + coff + cw],
                                start=(st == 0),
                                stop=(st == n_st - 1),
                            )
                        if ci == 0:
                            nc.vector.reciprocal(rinv[:], op[:, 0:1])
                            nc.vector.tensor_scalar_mul(
                                ob[:, 0 : cw - 1], op[:, 1:cw], rinv[:]
                            )
                        else:
                            nc.vector.tensor_scalar_mul(
                                ob[:, coff - 1 : coff - 1 + cw], op, rinv[:]
                            )
                        # store each chunk as soon as it is normalized: the
                        # final tile drains during the last chunks' matmuls
                        lo = 0 if ci == 0 else coff - 1
                        hi = coff - 1 + cw
                        qrow = (b * qt_per_qb + qs) * P
                        nc.sync.dma_start(
                            o_ap[qrow : qrow + P, lo:hi], ob[:, lo:hi]
                        )

    nc.compile()
    return nc


_cached_nc = None


def _get_nc():
    global _cached_nc
    if _cached_nc is None:
        _cached_nc = _build(LQ_SH, LS, H)
    return _cached_nc


def _in_maps(query_input, source_input):
    import ml_dtypes

    bf16 = ml_dtypes.bfloat16
    q = np.asarray(query_input, dtype=np.float32).astype(bf16)
    s = np.asarray(source_input, dtype=np.float32).astype(bf16)
    assert q.shape == (B, LQ, H) and s.shape == (B, LS, H)
    in_maps = []
    for c in range(N_CORES):
        b, qh = divmod(c, Q_SPLIT)
        in_maps.append(
            {
                "q_t": np.ascontiguousarray(
                    q[b, qh * LQ_SH : (qh + 1) * LQ_SH, :].T
                ),
                "source_input": np.ascontiguousarray(s[b]),
                "s_t": np.ascontiguousarray(s[b].T),
            }
        )
    return in_maps


def _gather(results):
    out = np.empty((B, LQ, H), dtype=np.float32)
    for c in range(N_CORES):
        b, qh = divmod(c, Q_SPLIT)
        out[b, qh * LQ_SH : (qh + 1) * LQ_SH, :] = results[c]["out"]
    return out


def kernel(query_input, source_input):
    from concourse.bass_utils import run_bass_kernel_spmd

    res = run_bass_kernel_spmd(
        _get_nc(),
        _in_maps(query_input, source_input),
        core_ids=list(range(N_CORES)),
    )
    return _gather(res.results)


# revision 19
# speedup vs baseline: 1.2800x; 1.0066x over previous
"""Trainium2 Bass kernel: batched dense attention (softmax(Q S^T / sqrt(H)) S).

Full problem: query [4, 4096, 1024], source [4, 4096, 1024] (source doubles
as values), output [4, 4096, 1024], all float32.

Sharding: pure data parallel over 8 NeuronCores — core c handles batch
c//2, query rows (c%2)*2048 ... +2048 with the full source for that batch
replicated to the core host-side.  No collectives are needed.

The host pre-casts Q and S to bf16 (the on-chip compute dtype), halving
staging DMA and removing all on-chip casts.

Per-core kernel (transposed-P1 formulation, bf16 matmuls, f32 PSUM).  The
PE does pure matmul work — no PE transposes at all:
  - S^T staged by 8 DRAM->SBUF xbar DMA-transposes; Q^T by one xbar
    DMA-transpose per 512-query block.  All xbar transposes ride the same
    (ACT) HWDGE ring — the xbar destination base is engine register state,
    so transposes on different rings corrupt each other.
  - S natural layout staged with a leading all-ones column per source
    tile ([128, 1+h] each), so P2 computes softmax row sums for free.
  - per 512-query block:
      P1T: logitsT tiles [128s, 512q] = S^T_tile.T @ Q^T  (contract over H)
      exp on ACT with scale=1/32 -> W^T tiles [128s, 512q] in SBUF, which
      are directly P2's stationary operand (this is why P1 is transposed:
      no W transpose is ever needed, and no softmax-sum accumulation).
      P2 per 128-query tile: O'[q, {R, h}] += W^T.T @ [1|S] over 3 PSUM
      chunks (342+342+341 cols); chunk 0 col 0 is the row sum R.
      Normalize by 1/R on DVE while writing bf16 output.
"""

import math

import numpy as np

B, LQ, LS, H = 4, 4096, 4096, 1024
N_CORES = 8
Q_SPLIT = 2  # query-length split within each batch entry
LQ_SH = LQ // Q_SPLIT  # 2048 query rows per core

P = 128  # partitions
QB = 512  # query-block width (P1T moving columns)
SC = 512  # S^T staging chunk width (s columns per xbar transpose)
N_WARM = 16  # dummy matmuls to promote the PE HAM clock gate at t=0
# P2 output chunking: 1 (ones) + 1024 (h) columns in 3 PSUM chunks
P2_CHUNKS = ((0, 342), (342, 342), (684, 341))


def _build(lq_sh, ls, h):
    """Build + compile the per-core Bass graph for shard shapes."""
    import concourse.bacc as bacc
    import concourse.mybir as mybir
    import concourse.tile as tile

    f32 = mybir.dt.float32
    bf16 = mybir.dt.bfloat16

    n_st = ls // P  # source tiles
    n_hc = h // P  # h chunks (contraction tiles for P1T)
    sc = min(SC, ls)
    n_sc = ls // sc  # S^T staging chunks
    st_per_sc = sc // P
    qb = min(QB, lq_sh)
    n_qb = lq_sh // qb  # query blocks
    qt_per_qb = qb // P
    w = h + 1  # per-source-tile staged width (ones column + h)
    scale = 1.0 / math.sqrt(h)

    nc = bacc.Bacc(
        "TRN2",
        target_bir_lowering=False,
        debug=False,
        num_devices=N_CORES,
    )
    qt_h = nc.dram_tensor("q_t", [h, lq_sh], bf16, kind="ExternalInput")
    s_h = nc.dram_tensor("source_input", [ls, h], bf16, kind="ExternalInput")
    st_h = nc.dram_tensor("s_t", [h, ls], bf16, kind="ExternalInput")
    o_h = nc.dram_tensor("out", [lq_sh, h], bf16, kind="ExternalOutput")
    s_ap, o_ap = s_h.ap(), o_h.ap()
    # [h, n] DRAM views as [p, hc, n]: row hc*P + p
    qt_ap3 = qt_h.ap().rearrange("(hc p) n -> p hc n", p=P)
    st_ap3 = st_h.ap().rearrange("(hc p) n -> p hc n", p=P)

    with tile.TileContext(nc) as tc:
        from contextlib import ExitStack

        with ExitStack() as ctx:
            # PE clock warmup: a dense burst of junk matmuls at t=0 fills the
            # HAM activity window so the 2.4 GHz clock engages before real
            # matmul work arrives (and mainline density then keeps it warm).
            warm_pool = ctx.enter_context(tc.tile_pool(name="warm", bufs=1))
            warm_w = warm_pool.tile([P, P], bf16)
            warm_x = warm_pool.tile([P, qb], bf16)
            nc.vector.memset(warm_w[:], 0.0)
            nc.vector.memset(warm_x[:], 0.0)
            psum_lg = ctx.enter_context(
                tc.tile_pool(name="psum_lg", bufs=4, space="PSUM")
            )
            wp = psum_lg.tile([P, qb], f32, tag="lgT", name="warmpsum")
            for _ in range(N_WARM):
                nc.tensor.matmul(wp[:], warm_w[:], warm_x[:], start=True, stop=True)

            qT_pool = ctx.enter_context(tc.tile_pool(name="qT", bufs=2))
            qTs = {}

            def issue_qT(b):
                # qT[p, hc, j] = Q^T[hc*P + p, b*qb + j], plain DMA
                t = qT_pool.tile([P, n_hc * qb], bf16, tag="qT")
                nc.sync.dma_start(
                    t.rearrange("p (hc j) -> p hc j", j=qb),
                    qt_ap3[:, :, b * qb : (b + 1) * qb],
                )
                qTs[b] = t

            issue_qT(0)

            persist = ctx.enter_context(tc.tile_pool(name="persist", bufs=1))
            # S^T in n_sc chunks; chunk sci holds h-chunk hc at cols
            # [hc*sc, +sc): sT[p, hc*sc + j] = S[sci*sc + j, hc*P + p].
            # Chunk 0 is staged as st_per_sc per-source-tile tiles instead, so
            # the first P1T chain's stationary (256 KB) lands ~4us earlier
            # than a whole 1 MiB chunk would.
            s_T2 = [
                persist.tile([P, n_hc * P], bf16, tag=f"sT2_{i}", name=f"sT2_{i}")
                for i in range(st_per_sc)
            ]
            s_T = {
                i: persist.tile([P, n_hc * sc], bf16, tag=f"sT{i}", name=f"sT{i}")
                for i in range(1, n_sc)
            }
            for st in range(st_per_sc):
                nc.sync.dma_start(
                    s_T2[st].rearrange("p (hc j) -> p hc j", j=P),
                    st_ap3[:, :, st * P : (st + 1) * P],
                )
            for sci in range(1, n_sc):
                nc.sync.dma_start(
                    s_T[sci].rearrange("p (hc j) -> p hc j", j=sc),
                    st_ap3[:, :, sci * sc : (sci + 1) * sc],
                )

            # S natural layout with a leading ones column per source tile:
            # tile st at cols [st*w, +w): col 0 = 1.0, cols 1..h = S[st*P+p, :].
            s_nat = persist.tile([P, n_st * w], bf16)
            nc.vector.memset(
                s_nat.rearrange("p (st c) -> p st c", c=w)[:, :, 0:1], 1.0
            )
            for st in range(n_st):
                nc.sync.dma_start(
                    s_nat[:, st * w + 1 : (st + 1) * w],
                    s_ap[st * P : (st + 1) * P, :],
                )

            # W^T tiles: one [128s, qb] tile per source tile, written by ACT
            # exp directly (the transposed-P1 trick).  Single-buffered: the
            # PE's own P2(k) -> P1T(k+1) ordering provides the reuse window.
            wT_pool = ctx.enter_context(tc.tile_pool(name="wT", bufs=1))
            psum_o = ctx.enter_context(
                tc.tile_pool(name="psum_o", bufs=4, space="PSUM")
            )
            r_pool = ctx.enter_context(tc.tile_pool(name="r", bufs=8))
            osb_pool = ctx.enter_context(tc.tile_pool(name="osb", bufs=3))

            for b in range(n_qb):
                if b + 1 < n_qb:
                    issue_qT(b + 1)
                qT = qTs.pop(b)
                wT = [
                    wT_pool.tile([P, qb], bf16, tag=f"wt{st}", name=f"wt{st}")
                    for st in range(n_st)
                ]
                # P1T: logitsT tiles, one per source tile
                for st in range(n_st):
                    sci, soff = divmod(st, st_per_sc)
                    lgT = psum_lg.tile([P, qb], f32, tag="lgT")
                    for hc in range(n_hc):
                        if sci == 0:
                            stat = s_T2[st][:, hc * P : (hc + 1) * P]
                        else:
                            stat = s_T[sci][
                                :, hc * sc + soff * P : hc * sc + (soff + 1) * P
                            ]
                        nc.tensor.matmul(
                            lgT[:],
                            stat,
                            qT[:, hc * qb : (hc + 1) * qb],
                            start=(hc == 0),
                            stop=(hc == n_hc - 1),
                        )
                    nc.scalar.activation(
                        wT[st][:],
                        lgT[:],
                        mybir.ActivationFunctionType.Exp,
                        scale=scale,
                    )

                # P2 per 128-query tile: 3 chunks over [1|S]; chunk 0 col 0
                # accumulates the softmax row sum.
                for qs in range(qt_per_qb):
                    ob = osb_pool.tile([P, h], bf16, tag="ob")
                    rinv = r_pool.tile([P, 1], f32, tag="rinv")
                    for ci, (coff, cw) in enumerate(P2_CHUNKS):
                        opt = psum_o.tile([P, P2_CHUNKS[0][1]], f32, tag="op", name="op")
                        op = opt[:, :cw]
                        for st in range(n_st):
                            nc.tensor.matmul(
                                op,
                                wT[st][:, qs * P : (qs + 1) * P],
                                s_nat[:, st * w + coff : st * w + coff + cw],
                                start=(st == 0),
                                stop=(st == n_st - 1),
                            )
                        if ci == 0:
                            nc.vector.reciprocal(rinv[:], op[:, 0:1])
                            nc.vector.tensor_scalar_mul(
                                ob[:, 0 : cw - 1], op[:, 1:cw], rinv[:]
                            )
                        else:
                            nc.vector.tensor_scalar_mul(
                                ob[:, coff - 1 : coff - 1 + cw], op, rinv[:]
                            )
                        # store each chunk as soon as it is normalized: the
                        # final tile drains during the last chunks' matmuls
                        lo = 0 if ci == 0 else coff - 1
                        hi = coff - 1 + cw
                        qrow = (b * qt_per_qb + qs) * P
                        nc.sync.dma_start(
                            o_ap[qrow : qrow + P, lo:hi], ob[:, lo:hi]
                        )

    nc.compile()
    return nc


_cached_nc = None


def _get_nc():
    global _cached_nc
    if _cached_nc is None:
        _cached_nc = _build(LQ_SH, LS, H)
    return _cached_nc


def _in_maps(query_input, source_input):
    import ml_dtypes

    bf16 = ml_dtypes.bfloat16
    q = np.asarray(query_input, dtype=np.float32).astype(bf16)
    s = np.asarray(source_input, dtype=np.float32).astype(bf16)
    assert q.shape == (B, LQ, H) and s.shape == (B, LS, H)
    in_maps = []
    for c in range(N_CORES):
        b, qh = divmod(c, Q_SPLIT)
        in_maps.append(
            {
                "q_t": np.ascontiguousarray(
                    q[b, qh * LQ_SH : (qh + 1) * LQ_SH, :].T
                ),
                "source_input": np.ascontiguousarray(s[b]),
                "s_t": np.ascontiguousarray(s[b].T),
            }
        )
    return in_maps


def _gather(results):
    out = np.empty((B, LQ, H), dtype=np.float32)
    for c in range(N_CORES):
        b, qh = divmod(c, Q_SPLIT)
        out[b, qh * LQ_SH : (qh + 1) * LQ_SH, :] = results[c]["out"]
    return out


def kernel(query_input, source_input):
    from concourse.bass_utils import run_bass_kernel_spmd

    res = run_bass_kernel_spmd(
        _get_nc(),
        _in_maps(query_input, source_input),
        core_ids=list(range(N_CORES)),
    )
    return _gather(res.results)
